# revision 14
# baseline (speedup 1.0000x reference)
"""Trainium2 Bass kernel for EnhancedMemoryEfficientAttention.

Sharding: 8 cores, core i owns spatial chunk i (2048 of 16384 positions)
for all 4 heads (each 2048x2048 attention block is independent per
(head, chunk)).  SE mean and GroupNorm stats are global -> computed from
one 66KB AllReduce of (s, C) where s = rowsum(attn_out), C = attn_out @
attn_out^T; SE gating and GN mu/var are derived analytically from (s, C)
so only a single collective sync is needed.

Key softmax trick: the module contracts the *unnormalized* axis
(out = V @ softmax(S)), so 1/Z folds into V^T columns ([128,32] scale per
strip) instead of a full [128,2048] pass, and exp's accum_out gives Z for
free on the ScalarE.
"""
import numpy as np

HEADS, DH, CHUNK, GROUPS, EPS = 4, 32, 2048, 8, 1e-5
INNER = HEADS * DH          # 128
HW = 16384
NCORES = 8
NSTRIP = CHUNK // 128       # 16 c-strips per block
SCALE = DH ** -0.5

# packed f32 consts column layout: [128, NCONST]
_IDENT = (0, 128)       # identity for PE transpose
_OUTWT = (128, 256)     # out_w^T
_SEW1T = (256, 288)     # se_w1^T  [128, 32]
_SEW2T = (288, 416)     # se_w2^T  in rows 0:32
_GMASK = (416, 424)     # group mean mask [128, 8] (1/16)
_GMASKT = (424, 552)    # group broadcast mask in rows 0:8
_SEB1 = 552             # se_b1 in rows 0:32
_SEB2 = 553
_OUTB = 554
_OUTB2X = 555
_OUTBSQ = 556
_GNW = 557
_GNB = 558
_ONESHW = 559
NCONST = 560

_NC = None


def _build(variant="full"):
    from contextlib import ExitStack

    import concourse.bacc as bacc
    import concourse.tile as tile
    from concourse import mybir

    f32 = mybir.dt.float32
    bf16 = mybir.dt.bfloat16
    AF = mybir.ActivationFunctionType
    OP = mybir.AluOpType

    nc = bacc.Bacc("TRN2", target_bir_lowering=False, debug=False,
                   num_devices=NCORES)

    x_d = nc.dram_tensor("x", [128, CHUNK], bf16, kind="ExternalInput").ap()
    wqkv_d = nc.dram_tensor("wqkv", [128, 3 * 128], bf16,
                            kind="ExternalInput").ap()
    cst_d = nc.dram_tensor("consts", [128, NCONST], f32,
                           kind="ExternalInput").ap()
    out_d = nc.dram_tensor("out", [128, CHUNK], f32, kind="ExternalOutput").ap()

    if variant == "nullio":
        with tile.TileContext(nc) as tc:
            with tc.tile_pool(name="sb", bufs=1) as sb:
                t = sb.tile([128, 16], f32, tag="t")
                nc.sync.dma_start(out=t, in_=cst_d[:, 0:16])
                nc.sync.dma_start(out=out_d[:, 0:16], in_=t)
        nc.compile()
        return nc

    with tile.TileContext(nc) as tc, ExitStack() as ctx:
        const = ctx.enter_context(tc.tile_pool(name="const", bufs=1))
        big = ctx.enter_context(tc.tile_pool(name="big", bufs=1))
        apool = ctx.enter_context(tc.tile_pool(name="apool", bufs=6))
        vpool = ctx.enter_context(tc.tile_pool(name="vpool", bufs=3))
        zpool = ctx.enter_context(tc.tile_pool(name="zpool", bufs=4))
        small = ctx.enter_context(tc.tile_pool(name="small", bufs=2))
        # PSUM budget: spsum 2 slots x [128,1024]f32 (2 banks) = 4 banks,
        # opsum 1 slot x [128,2048]f32 = 4 banks  -> 8 banks total.
        spsum = ctx.enter_context(tc.tile_pool(name="spsum", bufs=2, space="PSUM"))
        opsum = ctx.enter_context(tc.tile_pool(name="opsum", bufs=1, space="PSUM"))
        dram = ctx.enter_context(tc.tile_pool(name="dram", bufs=1, space="DRAM"))

        # ---- load inputs to SBUF (3 DMAs; x split so qkv can start early)
        cst = const.tile([128, NCONST], f32, tag="cst")
        nc.sync.dma_start(out=cst, in_=cst_d)
        wqkv = const.tile([128, 3 * 128], bf16, tag="wqkv")
        nc.sync.dma_start(out=wqkv, in_=wqkv_d)
        xw = const.tile([128, CHUNK], bf16, tag="x")
        for xq in range(4):
            nc.sync.dma_start(out=xw[:, xq * 512:(xq + 1) * 512],
                              in_=x_d[:, xq * 512:(xq + 1) * 512])

        wq, wk, wv = wqkv[:, 0:128], wqkv[:, 128:256], wqkv[:, 256:384]
        ident = cst[:, _IDENT[0]:_IDENT[1]]
        out_w_t = cst[:, _OUTWT[0]:_OUTWT[1]]
        se_w1_t = cst[:, _SEW1T[0]:_SEW1T[1]]
        se_w2_t = cst[0:32, _SEW2T[0]:_SEW2T[1]]
        gmask = cst[:, _GMASK[0]:_GMASK[1]]
        gmask_t = cst[0:GROUPS, _GMASKT[0]:_GMASKT[1]]
        se_b1 = cst[0:32, _SEB1:_SEB1 + 1]
        se_b2 = cst[:, _SEB2:_SEB2 + 1]
        out_b = cst[:, _OUTB:_OUTB + 1]
        out_b_2x = cst[:, _OUTB2X:_OUTB2X + 1]
        out_b_sq = cst[:, _OUTBSQ:_OUTBSQ + 1]
        gn_w = cst[:, _GNW:_GNW + 1]
        gn_b = cst[:, _GNB:_GNB + 1]
        ones_hw = cst[:, _ONESHW:_ONESHW + 1]

        # ---- qkv: q = Wq @ x, k = Wk @ x (inner on partitions, bf16)
        q_sb = big.tile([128, CHUNK], bf16, tag="q")
        k_sb = big.tile([128, CHUNK], bf16, tag="k")
        vt_sb = big.tile([128, CHUNK], f32, tag="vt")  # [spatial, inner]/chunk
        for half in range(2):
            for w_sb, dst in ((wq, q_sb), (wk, k_sb)):
                ps = spsum.tile([128, 1024], f32, tag="s")
                for j in range(2):
                    c0 = half * 1024 + j * 512
                    nc.tensor.matmul(ps[:, j * 512:(j + 1) * 512],
                                     lhsT=w_sb, rhs=xw[:, c0:c0 + 512],
                                     start=True, stop=True)
                nc.vector.tensor_copy(
                    out=dst[:, half * 1024:(half + 1) * 1024], in_=ps)
        # v^T directly: vT[p, d] = sum_cin x[cin, p] * WvT[cin, d]
        for ci in range(NSTRIP):
            ps = spsum.tile([128, 128], f32, tag="s")
            nc.tensor.matmul(ps, lhsT=xw[:, ci * 128:(ci + 1) * 128], rhs=wv,
                             start=True, stop=True)
            nc.vector.tensor_copy(out=vt_sb[:, ci * 128:(ci + 1) * 128], in_=ps)

        if variant == "qkv":
            nc.sync.dma_start(out=out_d, in_=vt_sb)
            nc.compile()
            return nc

        # ---- attention main loop
        rep_main = 4 if variant == "rep4" else 1
        o_ps = opsum.tile([128, CHUNK], f32, tag="o")
        out_sb = big.tile([128, CHUNK], f32, tag="outsb")
        s_part = small.tile([128, 1], f32, tag="spart")
        for h in range(HEADS * rep_main):
            h = h % HEADS
            hb = slice(32 * h, 32 * h + 32)
            for ci in range(NSTRIP):
                a_tiles = []
                zs = []
                for eh in range(2):
                    s_ps = spsum.tile([128, 1024], f32, tag="s")
                    for j in range(2):
                        e0 = eh * 1024 + j * 512
                        for _ in range(2 if variant == "sx2" else 1):
                            nc.tensor.matmul(
                                s_ps[:, j * 512:(j + 1) * 512],
                                lhsT=q_sb[hb, ci * 128:(ci + 1) * 128],
                                rhs=k_sb[hb, e0:e0 + 512],
                                start=True, stop=True,
                                tile_position=(32 * h, 0))
                    a_t = apool.tile([128, 1024], bf16, tag="a")
                    z_t = zpool.tile([128, 1], f32, tag="z")
                    for _ in range(2 if variant == "expx2" else 1):
                        nc.scalar.activation(out=a_t, in_=s_ps, func=AF.Exp,
                                             scale=SCALE, accum_out=z_t)
                    a_tiles.append(a_t)
                    zs.append(z_t)
                zsum = zpool.tile([128, 1], f32, tag="zsum")
                nc.vector.tensor_add(out=zsum, in0=zs[0], in1=zs[1])
                rz = zpool.tile([128, 1], f32, tag="rz")
                nc.vector.reciprocal(out=rz, in_=zsum)
                # V''^T strip: V^T columns of head h scaled by 1/Z (per row)
                vtt = vpool.tile([128, 32], bf16, tag="vtt")
                nc.vector.tensor_scalar_mul(
                    out=vtt,
                    in0=vt_sb[:, ci * 128 + 32 * h: ci * 128 + 32 * h + 32],
                    scalar1=rz)
                for q4 in range(4):
                    a_t = a_tiles[q4 // 2]
                    for _ in range(2 if variant == "vax2" else 1):
                        nc.tensor.matmul(
                            o_ps[hb, q4 * 512:(q4 + 1) * 512],
                            lhsT=vtt,
                            rhs=a_t[:, (q4 % 2) * 512:(q4 % 2 + 1) * 512],
                            start=(ci == 0), stop=(ci == NSTRIP - 1),
                            tile_position=(0, 32 * h),
                            skip_group_check=True)
            # evacuate this head's O band; accum gives s partial rowsums
            nc.vector.tensor_scalar(
                out=out_sb[hb, :], in0=o_ps[hb, :],
                scalar1=1.0, scalar2=0.0, op0=OP.mult, op1=OP.add,
                accum_out=s_part[hb, :])

        if variant == "attn":
            nc.sync.dma_start(out=out_d, in_=out_sb)
            nc.compile()
            return nc

        # ---- C = out @ out^T (local partial) via PE transposes
        outT_sb = big.tile([128, CHUNK], f32, tag="outT")
        c_ps = opsum.tile([128, 128], f32, tag="o")  # reuses o slot
        for rep in range(4 if variant == "cx4" else 1):
            for ci in range(NSTRIP):
                t_ps = spsum.tile([128, 128], f32, tag="s")
                nc.tensor.transpose(t_ps, out_sb[:, ci * 128:(ci + 1) * 128],
                                    ident)
                nc.vector.tensor_copy(out=outT_sb[:, ci * 128:(ci + 1) * 128],
                                      in_=t_ps)
                nc.tensor.matmul(c_ps,
                                 lhsT=outT_sb[:, ci * 128:(ci + 1) * 128],
                                 rhs=outT_sb[:, ci * 128:(ci + 1) * 128],
                                 start=(ci == 0), stop=(ci == NSTRIP - 1),
                                 skip_group_check=True)
        c_sb = small.tile([128, 128], f32, tag="csb")
        nc.vector.tensor_copy(out=c_sb, in_=c_ps)

        # ---- single AllReduce of [s | C]  (128 x 129 f32 = 66KB)
        cc_in = dram.tile([128, 129], f32, tag="ccin")
        cc_out = dram.tile([128, 129], f32, tag="ccout")
        nc.sync.dma_start(out=cc_in[:, 0:1], in_=s_part)
        nc.sync.dma_start(out=cc_in[:, 1:129], in_=c_sb)
        sc_sb = small.tile([128, 129], f32, tag="scsb")
        if variant == "noc":
            nc.sync.dma_start(out=sc_sb, in_=cc_in)
        else:
            for _ in range(4 if variant == "cc4" else 1):
                nc.gpsimd.collective_compute(
                    "AllReduce", OP.add,
                    replica_groups=[list(range(NCORES))],
                    ins=[cc_in.opt()], outs=[cc_out.opt()])
            nc.sync.dma_start(out=sc_sb, in_=cc_out)

        for rep in range(2 if variant == "tailx2" else 1):
            # ---- SE gating (from global s)
            s_mean = small.tile([128, 1], f32, tag="smean")
            nc.vector.tensor_scalar_mul(out=s_mean, in0=sc_sb[:, 0:1],
                                        scalar1=1.0 / HW)
            z1_ps = spsum.tile([32, 1], f32, tag="s")
            nc.tensor.matmul(z1_ps, lhsT=se_w1_t, rhs=s_mean,
                             start=True, stop=True)
            z1_sb = small.tile([32, 1], f32, tag="z1")
            nc.scalar.activation(out=z1_sb, in_=z1_ps, func=AF.Silu,
                                 bias=se_b1)
            g_ps = spsum.tile([128, 1], f32, tag="s")
            nc.tensor.matmul(g_ps, lhsT=se_w2_t, rhs=z1_sb,
                             start=True, stop=True)
            g_sb = small.tile([128, 1], f32, tag="g")
            nc.scalar.activation(out=g_sb, in_=g_ps, func=AF.Sigmoid,
                                 bias=se_b2)

            # W'^T = out_w^T * g  (per-partition over cin=inner)
            wp_sb = small.tile([128, 128], f32, tag="wp")
            nc.vector.tensor_scalar_mul(out=wp_sb, in0=out_w_t, scalar1=g_sb)

            # ---- y = W' @ out (bias/GN affine folded into final activation)
            y_ps = opsum.tile([128, CHUNK], f32, tag="o")
            for jc in range(4):
                nc.tensor.matmul(y_ps[:, jc * 512:(jc + 1) * 512],
                                 lhsT=wp_sb,
                                 rhs=out_sb[:, jc * 512:(jc + 1) * 512],
                                 start=True, stop=True)

            # ---- GroupNorm stats from (s, C):
            # E_p[y_o^2] = w'_o (C/HW) w'_o^T + 2 b_o (w'_o.s_mean) + b_o^2
            u_ps = spsum.tile([128, 128], f32, tag="s")
            nc.tensor.matmul(u_ps, lhsT=sc_sb[:, 1:129], rhs=wp_sb,
                             start=True, stop=True)
            u_sb = small.tile([128, 128], f32, tag="usb")
            nc.vector.tensor_copy(out=u_sb, in_=u_ps)
            v1_sb = small.tile([128, 128], f32, tag="v1")
            nc.vector.tensor_mul(out=v1_sb, in0=wp_sb, in1=u_sb)
            e2_ps = spsum.tile([128, 1], f32, tag="s")
            nc.tensor.matmul(e2_ps, lhsT=v1_sb, rhs=ones_hw,
                             start=True, stop=True)
            mu_ps = spsum.tile([128, 1], f32, tag="s")
            nc.tensor.matmul(mu_ps, lhsT=wp_sb, rhs=s_mean,
                             start=True, stop=True)

            stats = small.tile([128, 2], f32, tag="stats")
            nc.vector.tensor_scalar_add(out=stats[:, 0:1], in0=mu_ps,
                                        scalar1=out_b)
            t1 = small.tile([128, 1], f32, tag="t1")
            nc.vector.tensor_scalar_mul(out=t1, in0=mu_ps, scalar1=out_b_2x)
            t2 = small.tile([128, 1], f32, tag="t2")
            nc.vector.tensor_add(out=t2, in0=e2_ps, in1=t1)
            nc.vector.tensor_scalar_add(out=stats[:, 1:2], in0=t2,
                                        scalar1=out_b_sq)

            gm_ps = spsum.tile([GROUPS, 2], f32, tag="s")
            nc.tensor.matmul(gm_ps, lhsT=gmask, rhs=stats,
                             start=True, stop=True)
            gm_sb = small.tile([GROUPS, 2], f32, tag="gm")
            nc.vector.tensor_copy(out=gm_sb, in_=gm_ps)
            m2 = small.tile([GROUPS, 1], f32, tag="m2")
            nc.vector.tensor_mul(out=m2, in0=gm_sb[:, 0:1], in1=gm_sb[:, 0:1])
            var = small.tile([GROUPS, 1], f32, tag="var")
            nc.vector.tensor_sub(out=var, in0=gm_sb[:, 1:2], in1=m2)
            eps_t = small.tile([GROUPS, 1], f32, tag="eps")
            nc.vector.memset(eps_t, EPS)
            sq = small.tile([GROUPS, 1], f32, tag="sq")
            nc.scalar.activation(out=sq, in_=var, func=AF.Sqrt, bias=eps_t)
            rsq = small.tile([GROUPS, 1], f32, tag="rsq")
            nc.vector.reciprocal(out=rsq, in_=sq)

            rm = small.tile([GROUPS, 2], f32, tag="rm")
            nc.vector.tensor_copy(out=rm[:, 0:1], in_=rsq)
            nc.vector.tensor_copy(out=rm[:, 1:2], in_=gm_sb[:, 0:1])
            bc_ps = spsum.tile([128, 2], f32, tag="s")
            nc.tensor.matmul(bc_ps, lhsT=gmask_t, rhs=rm, start=True, stop=True)
            bc_sb = small.tile([128, 2], f32, tag="bc")
            nc.vector.tensor_copy(out=bc_sb, in_=bc_ps)

            # alpha = rsq_o * gn_w ; beta = gn_b - alpha*(mu_o - out_b)
            alpha = small.tile([128, 1], f32, tag="alpha")
            nc.vector.tensor_scalar_mul(out=alpha, in0=bc_sb[:, 0:1],
                                        scalar1=gn_w)
            t3 = small.tile([128, 1], f32, tag="t3")
            nc.vector.scalar_tensor_tensor(out=t3, in0=bc_sb[:, 1:2],
                                           scalar=out_b, in1=alpha,
                                           op0=OP.subtract, op1=OP.mult)
            beta = small.tile([128, 1], f32, tag="beta")
            nc.vector.tensor_scalar(out=beta, in0=t3, scalar1=-1.0,
                                    scalar2=gn_b, op0=OP.mult, op1=OP.add)

            # ---- final affine, chunked so output DMA overlaps ScalarE
            yn_sb = big.tile([128, CHUNK], f32, tag="yn")
            for jc in range(4):
                sl = slice(jc * 512, (jc + 1) * 512)
                nc.scalar.activation(out=yn_sb[:, sl], in_=y_ps[:, sl],
                                     func=AF.Identity, bias=beta, scale=alpha)
                nc.sync.dma_start(out=out_d[:, sl], in_=yn_sb[:, sl])

    nc.compile()
    return nc


def _get_nc():
    global _NC
    if _NC is None:
        _NC = _build()
    return _NC


def _host_inputs(x, w_qkv, se_w1, se_b1, se_w2, se_b2, out_w, out_b,
                 gn_w, gn_b):
    import ml_dtypes
    bf = ml_dtypes.bfloat16
    f32 = np.float32

    def c(a, dt=f32):
        return np.ascontiguousarray(np.asarray(a), dtype=dt)

    x2 = np.asarray(x, dtype=f32).reshape(INNER, HW)
    w_qkv = np.asarray(w_qkv, dtype=f32)
    out_b = np.asarray(out_b, dtype=f32)

    wqkv = c(np.concatenate(
        [w_qkv[0:128].T, w_qkv[128:256].T, w_qkv[256:384].T], axis=1), bf)

    gs = 128 // GROUPS
    cst = np.zeros((128, NCONST), f32)
    cst[:, _IDENT[0]:_IDENT[1]] = np.eye(128, dtype=f32)
    cst[:, _OUTWT[0]:_OUTWT[1]] = np.asarray(out_w, dtype=f32).T
    cst[:, _SEW1T[0]:_SEW1T[1]] = np.asarray(se_w1, dtype=f32).T
    cst[0:32, _SEW2T[0]:_SEW2T[1]] = np.asarray(se_w2, dtype=f32).T
    cst[np.arange(128), _GMASK[0] + np.arange(128) // gs] = 1.0 / gs
    cst[np.arange(128) // gs, _GMASKT[0] + np.arange(128)] = 1.0
    cst[0:32, _SEB1] = np.asarray(se_b1, dtype=f32)
    cst[:, _SEB2] = np.asarray(se_b2, dtype=f32)
    cst[:, _OUTB] = out_b
    cst[:, _OUTB2X] = 2.0 * out_b
    cst[:, _OUTBSQ] = out_b * out_b
    cst[:, _GNW] = np.asarray(gn_w, dtype=f32)
    cst[:, _GNB] = np.asarray(gn_b, dtype=f32)
    cst[:, _ONESHW] = 1.0 / HW

    in_maps = []
    for i in range(NCORES):
        in_maps.append({
            "x": c(x2[:, i * CHUNK:(i + 1) * CHUNK], bf),
            "wqkv": wqkv,
            "consts": cst,
        })
    return in_maps


def kernel(x, w_qkv, se_w1, se_b1, se_w2, se_b2, out_w, out_b, gn_w, gn_b):
    from concourse.bass_utils import run_bass_kernel_spmd

    in_maps = _host_inputs(x, w_qkv, se_w1, se_b1, se_w2, se_b2, out_w,
                           out_b, gn_w, gn_b)
    res = run_bass_kernel_spmd(_get_nc(), in_maps, core_ids=list(range(NCORES)))
    y = np.concatenate([np.asarray(res.results[i]["out"], dtype=np.float32)
                        for i in range(NCORES)], axis=1)
    B, C, H, W = 1, 128, 128, 128
    return y.reshape(B, C, H, W)


# revision 29
# speedup vs baseline: 2.1291x; 2.1291x over previous
"""Trainium2 Bass kernel for EnhancedMemoryEfficientAttention.

Sharding: 8 cores, core i owns spatial chunk i (2048 of 16384 positions)
for all 4 heads (each 2048x2048 attention block is independent per
(head, chunk)).  SE mean and GroupNorm stats are global -> computed from
one 66KB AllReduce of (s, C) where s = rowsum(attn_out), C = attn_out @
attn_out^T; SE gating and GN mu/var are derived analytically from (s, C)
so only a single collective sync is needed.

Key softmax trick: the module contracts the *unnormalized* axis
(out = V @ softmax(S)), so 1/Z folds into V^T columns ([128,32] scale per
strip) instead of a full [128,2048] pass, and exp's accum_out gives Z for
free on the ScalarE.
"""
import numpy as np

HEADS, DH, CHUNK, GROUPS, EPS = 4, 32, 2048, 8, 1e-5
INNER = HEADS * DH          # 128
HW = 16384
NCORES = 8
NSTRIP = CHUNK // 128       # 16 c-strips per block
SCALE = DH ** -0.5

# packed f32 consts column layout: [128, NCONST]
_IDENT = (0, 128)       # identity for PE transpose
_OUTWT = (128, 256)     # out_w^T
_SEW1T = (256, 288)     # se_w1^T  [128, 32]
_SEW2T = (288, 416)     # se_w2^T  in rows 0:32
_GMASK = (416, 424)     # group mean mask [128, 8] (1/16)
_GMASKT = (424, 552)    # group broadcast mask in rows 0:8
_SEB1 = 552             # se_b1 in rows 0:32
_SEB2 = 553
_OUTB = 554
_OUTB2X = 555
_OUTBSQ = 556
_GNW = 557
_GNB = 558
_ONESHW = 559
_EPSC = 560
NCONST = 561

_NC = None


def _build(variant="full"):
    from contextlib import ExitStack

    import concourse.bacc as bacc
    import concourse.tile as tile
    from concourse import mybir

    f32 = mybir.dt.float32
    bf16 = mybir.dt.bfloat16
    AF = mybir.ActivationFunctionType
    OP = mybir.AluOpType

    nc = bacc.Bacc("TRN2", target_bir_lowering=False, debug=False,
                   num_devices=NCORES)

    x_d = nc.dram_tensor("x", [128, CHUNK], bf16, kind="ExternalInput").ap()
    wqkv_d = nc.dram_tensor("wqkv", [128, 3 * 128], bf16,
                            kind="ExternalInput").ap()
    cst_d = nc.dram_tensor("consts", [128, NCONST], f32,
                           kind="ExternalInput").ap()
    out_d = nc.dram_tensor("out", [128, CHUNK], f32, kind="ExternalOutput").ap()

    if variant == "nullio":
        with tile.TileContext(nc) as tc:
            with tc.tile_pool(name="sb", bufs=1) as sb:
                t = sb.tile([128, 16], f32, tag="t")
                nc.sync.dma_start(out=t, in_=cst_d[:, 0:16])
                nc.sync.dma_start(out=out_d[:, 0:16], in_=t)
        nc.compile()
        return nc

    with tile.TileContext(nc) as tc, ExitStack() as ctx:
        const = ctx.enter_context(tc.tile_pool(name="const", bufs=1))
        big = ctx.enter_context(tc.tile_pool(name="big", bufs=1))
        nbuf_a = 16 if variant.startswith("big_") else 12
        apool = ctx.enter_context(tc.tile_pool(name="apool", bufs=nbuf_a))
        vpool = ctx.enter_context(tc.tile_pool(name="vpool", bufs=5))
        zpool = ctx.enter_context(tc.tile_pool(name="zpool", bufs=10))
        small = ctx.enter_context(tc.tile_pool(name="small", bufs=2))
        # PSUM budget: spsum 2 slots x [128,1024]f32 (2 banks) = 4 banks,
        # opsum 1 slot x [128,2048]f32 = 4 banks  -> 8 banks total.
        spsum = ctx.enter_context(tc.tile_pool(name="spsum", bufs=2, space="PSUM"))
        opsum = ctx.enter_context(tc.tile_pool(name="opsum", bufs=1, space="PSUM"))
        dram = ctx.enter_context(tc.tile_pool(name="dram", bufs=1, space="DRAM"))

        # ---- load inputs to SBUF (3 DMAs; x split so qkv can start early)
        cst = const.tile([128, NCONST], f32, tag="cst")
        nc.sync.dma_start(out=cst, in_=cst_d)
        wqkv = const.tile([128, 3 * 128], bf16, tag="wqkv")
        nc.sync.dma_start(out=wqkv, in_=wqkv_d)
        xw = const.tile([128, CHUNK], bf16, tag="x")
        for xq in range(4):
            nc.sync.dma_start(out=xw[:, xq * 512:(xq + 1) * 512],
                              in_=x_d[:, xq * 512:(xq + 1) * 512])

        wq, wk, wv = wqkv[:, 0:128], wqkv[:, 128:256], wqkv[:, 256:384]
        ident = cst[:, _IDENT[0]:_IDENT[1]]
        out_w_t = cst[:, _OUTWT[0]:_OUTWT[1]]
        se_w1_t = cst[:, _SEW1T[0]:_SEW1T[1]]
        se_w2_t = cst[0:32, _SEW2T[0]:_SEW2T[1]]
        gmask = cst[:, _GMASK[0]:_GMASK[1]]
        gmask_t = cst[0:GROUPS, _GMASKT[0]:_GMASKT[1]]
        se_b1 = cst[0:32, _SEB1:_SEB1 + 1]
        se_b2 = cst[:, _SEB2:_SEB2 + 1]
        out_b = cst[:, _OUTB:_OUTB + 1]
        out_b_2x = cst[:, _OUTB2X:_OUTB2X + 1]
        out_b_sq = cst[:, _OUTBSQ:_OUTBSQ + 1]
        gn_w = cst[:, _GNW:_GNW + 1]
        gn_b = cst[:, _GNB:_GNB + 1]
        ones_hw = cst[:, _ONESHW:_ONESHW + 1]

        # ---- qkv: q = Wq @ x, k = Wk @ x (inner on partitions, bf16)
        q_sb = big.tile([128, CHUNK], bf16, tag="q")
        k_sb = big.tile([128, CHUNK], bf16, tag="k")
        vt_sb = big.tile([128, CHUNK], f32, tag="vt")  # [spatial, inner]/chunk
        for half in range(2):
            for wi, (w_sb, dst) in enumerate(((wq, q_sb), (wk, k_sb))):
                ps = spsum.tile([128, 1024], f32, tag="s")
                for j in range(2):
                    c0 = half * 1024 + j * 512
                    nc.tensor.matmul(ps[:, j * 512:(j + 1) * 512],
                                     lhsT=w_sb, rhs=xw[:, c0:c0 + 512],
                                     start=True, stop=True)
                if wi == 0:
                    nc.vector.tensor_copy(
                        out=dst[:, half * 1024:(half + 1) * 1024], in_=ps)
                else:
                    nc.scalar.copy(
                        out=dst[:, half * 1024:(half + 1) * 1024], in_=ps)
        # v^T directly: vT[p, d] = sum_cin x[cin, p] * WvT[cin, d]
        for ci in range(NSTRIP):
            ps = spsum.tile([128, 128], f32, tag="s")
            nc.tensor.matmul(ps, lhsT=xw[:, ci * 128:(ci + 1) * 128], rhs=wv,
                             start=True, stop=True)
            if ci % 2 == 0:
                nc.vector.tensor_copy(out=vt_sb[:, ci * 128:(ci + 1) * 128],
                                      in_=ps)
            else:
                nc.scalar.copy(out=vt_sb[:, ci * 128:(ci + 1) * 128], in_=ps)

        if variant == "qkv":
            nc.sync.dma_start(out=out_d, in_=vt_sb)
            nc.compile()
            return nc

        # ---- attention main loop
        # Per c-strip ci: 8 exps ([128,1024], 4 heads x 2 e-halves) with S
        # tiles double-buffered pair-wise in 2 PSUM slots; VA contributions
        # go to a transient per-strip PSUM tile (col-packed across heads)
        # and are accumulated into SBUF by the DVE, so ScalarE never waits.
        v_rep = variant[4:] if variant.startswith("big_") else variant
        rep_main = int(v_rep[3:]) if v_rep.startswith("rep") else 1
        out_sb = big.tile([128, CHUNK], f32, tag="outsb")
        s_part = small.tile([128, 1], f32, tag="spart")
        o_ps = opsum.tile([128, CHUNK], f32, tag="o")
        for ci_r in range(NSTRIP * rep_main):
            ci = ci_r % NSTRIP
            first = ci_r == 0
            a_tiles = {}
            zs = {}
            for eh in range(2):
                for h in range(HEADS):
                    hb = slice(32 * h, 32 * h + 32)
                    s_ps = spsum.tile([128, 1024], f32, tag="s")
                    for j in range(2):
                        e0 = eh * 1024 + j * 512
                        nc.tensor.matmul(
                            s_ps[:, j * 512:(j + 1) * 512],
                            lhsT=q_sb[hb, ci * 128:(ci + 1) * 128],
                            rhs=k_sb[hb, e0:e0 + 512],
                            start=True, stop=True,
                            tile_position=(32 * h, 0))
                    a_t = apool.tile([128, 1024], bf16, tag="a")
                    z_t = zpool.tile([128, 1], f32, tag="z")
                    nc.scalar.activation(out=a_t, in_=s_ps, func=AF.Exp,
                                         scale=SCALE, accum_out=z_t)
                    a_tiles[(h, eh)] = a_t
                    zs[(h, eh)] = z_t
            vtts = {}
            for h in range(HEADS):
                zsum = zpool.tile([128, 1], f32, tag="zsum")
                nc.vector.tensor_add(out=zsum, in0=zs[(h, 0)], in1=zs[(h, 1)])
                rz = zpool.tile([128, 1], f32, tag="rz")
                nc.vector.reciprocal(out=rz, in_=zsum)
                vtt = vpool.tile([128, 32], bf16, tag="vtt")
                nc.vector.tensor_scalar_mul(
                    out=vtt,
                    in0=vt_sb[:, ci * 128 + 32 * h: ci * 128 + 32 * h + 32],
                    scalar1=rz)
                vtts[h] = vtt
            # this strip's O contribution: col-packed heads, bursts of 4
            # col-groups run concurrently; accumulate across strips in PSUM
            for q4 in range(4):
                for h in range(HEADS):
                    nc.tensor.matmul(
                        o_ps[32 * h:32 * h + 32, q4 * 512:(q4 + 1) * 512],
                        lhsT=vtts[h],
                        rhs=a_tiles[(h, q4 // 2)][:, (q4 % 2) * 512:
                                                  (q4 % 2 + 1) * 512],
                        start=first, stop=(ci_r == NSTRIP * rep_main - 1),
                        tile_position=(0, 32 * h),
                        skip_group_check=True)
        # evacuate O from PSUM; the same op emits the s partial rowsums
        yn_sb = big.tile([128, CHUNK], f32, tag="yn")
        nc.vector.tensor_scalar(
            out=out_sb, in0=o_ps,
            scalar1=1.0, scalar2=0.0, op0=OP.mult, op1=OP.add,
            accum_out=s_part)
        # preload the sigmoid table set while C/collective run (dep on
        # s_part pins this after the last Exp)
        dummy1 = small.tile([128, 1], f32, tag="dummy1")
        nc.scalar.activation(out=dummy1, in_=s_part, func=AF.Sigmoid)

        if variant == "attn":
            nc.sync.dma_start(out=out_d, in_=out_sb)
            nc.compile()
            return nc

        # ---- C = out @ out^T (local partial) via PE transposes
        outT_sb = big.tile([128, CHUNK], f32, tag="outT")
        c_ps = opsum.tile([128, 128], f32, tag="o")  # reuses o slot
        for rep in range(int(variant[2:]) if variant.startswith("cx") else 1):
            for ci in range(NSTRIP):
                t_ps = spsum.tile([128, 128], f32, tag="s")
                nc.tensor.transpose(t_ps, out_sb[:, ci * 128:(ci + 1) * 128],
                                    ident)
                nc.vector.tensor_copy(out=outT_sb[:, ci * 128:(ci + 1) * 128],
                                      in_=t_ps)
                nc.tensor.matmul(c_ps,
                                 lhsT=outT_sb[:, ci * 128:(ci + 1) * 128],
                                 rhs=outT_sb[:, ci * 128:(ci + 1) * 128],
                                 start=(ci == 0), stop=(ci == NSTRIP - 1),
                                 skip_group_check=True)
        c_sb = small.tile([128, 128], f32, tag="csb")
        nc.vector.tensor_copy(out=c_sb, in_=c_ps)

        # ---- single AllReduce of [s | C]  (128 x 129 f32 = 66KB)
        cc_in = dram.tile([128, 129], f32, tag="ccin")
        cc_out = dram.tile([128, 129], f32, tag="ccout")
        nc.sync.dma_start(out=cc_in[:, 0:1], in_=s_part)
        nc.sync.dma_start(out=cc_in[:, 1:129], in_=c_sb)
        sc_sb = small.tile([128, 129], f32, tag="scsb")
        if variant == "noc":
            nc.sync.dma_start(out=sc_sb, in_=cc_in)
        else:
            for _ in range(int(variant[2:]) if variant.startswith("cc") else 1):
                nc.gpsimd.collective_compute(
                    "AllReduce", OP.add,
                    replica_groups=[list(range(NCORES))],
                    ins=[cc_in.opt()], outs=[cc_out.opt()])
            nc.sync.dma_start(out=sc_sb, in_=cc_out)

        for rep in range(int(variant[5:]) if variant.startswith("tailx") else 1):
            # ---- SE gating (from global s)
            s_mean = small.tile([128, 1], f32, tag="smean")
            nc.vector.tensor_scalar_mul(out=s_mean, in0=sc_sb[:, 0:1],
                                        scalar1=1.0 / HW)
            z1_ps = spsum.tile([32, 1], f32, tag="s")
            nc.tensor.matmul(z1_ps, lhsT=se_w1_t, rhs=s_mean,
                             start=True, stop=True)
            sg_sb = small.tile([32, 1], f32, tag="sg")
            nc.scalar.activation(out=sg_sb, in_=z1_ps, func=AF.Sigmoid,
                                 bias=se_b1)
            tb_sb = small.tile([32, 1], f32, tag="tb")
            nc.vector.tensor_scalar_add(out=tb_sb, in0=z1_ps, scalar1=se_b1)
            z1_sb = small.tile([32, 1], f32, tag="z1")
            nc.vector.tensor_mul(out=z1_sb, in0=tb_sb, in1=sg_sb)
            g_ps = spsum.tile([128, 1], f32, tag="s")
            nc.tensor.matmul(g_ps, lhsT=se_w2_t, rhs=z1_sb,
                             start=True, stop=True)
            g_sb = small.tile([128, 1], f32, tag="g")
            nc.scalar.activation(out=g_sb, in_=g_ps, func=AF.Sigmoid,
                                 bias=se_b2)

            dummy2 = small.tile([128, 1], f32, tag="dummy2")
            nc.scalar.activation(out=dummy2, in_=g_sb, func=AF.Sqrt)

            # W'^T = out_w^T * g  (per-partition over cin=inner)
            wp_sb = small.tile([128, 128], f32, tag="wp")
            nc.vector.tensor_scalar_mul(out=wp_sb, in0=out_w_t, scalar1=g_sb)

            # ---- y = W' @ out (bias/GN affine folded into final activation)
            y_ps = opsum.tile([128, CHUNK], f32, tag="o")
            for jc in range(4):
                nc.tensor.matmul(y_ps[:, jc * 512:(jc + 1) * 512],
                                 lhsT=wp_sb,
                                 rhs=out_sb[:, jc * 512:(jc + 1) * 512],
                                 start=True, stop=True)

            # ---- GroupNorm stats from (s, C):
            # E_p[y_o^2] = w'_o (C/HW) w'_o^T + 2 b_o (w'_o.s_mean) + b_o^2
            u_ps = spsum.tile([128, 128], f32, tag="s")
            nc.tensor.matmul(u_ps, lhsT=sc_sb[:, 1:129], rhs=wp_sb,
                             start=True, stop=True)
            u_sb = small.tile([128, 128], f32, tag="usb")
            nc.vector.tensor_copy(out=u_sb, in_=u_ps)
            v1_sb = small.tile([128, 128], f32, tag="v1")
            nc.vector.tensor_mul(out=v1_sb, in0=wp_sb, in1=u_sb)
            e2_ps = spsum.tile([128, 1], f32, tag="s")
            nc.tensor.matmul(e2_ps, lhsT=v1_sb, rhs=ones_hw,
                             start=True, stop=True)
            mu_ps = spsum.tile([128, 1], f32, tag="s")
            nc.tensor.matmul(mu_ps, lhsT=wp_sb, rhs=s_mean,
                             start=True, stop=True)

            stats = small.tile([128, 2], f32, tag="stats")
            nc.vector.tensor_scalar_add(out=stats[:, 0:1], in0=mu_ps,
                                        scalar1=out_b)
            t1 = small.tile([128, 1], f32, tag="t1")
            nc.vector.tensor_scalar_mul(out=t1, in0=mu_ps, scalar1=out_b_2x)
            t2 = small.tile([128, 1], f32, tag="t2")
            nc.vector.tensor_add(out=t2, in0=e2_ps, in1=t1)
            nc.vector.tensor_scalar_add(out=stats[:, 1:2], in0=t2,
                                        scalar1=out_b_sq)

            gm_ps = spsum.tile([GROUPS, 2], f32, tag="s")
            nc.tensor.matmul(gm_ps, lhsT=gmask, rhs=stats,
                             start=True, stop=True)
            gm_sb = small.tile([GROUPS, 2], f32, tag="gm")
            nc.vector.tensor_copy(out=gm_sb, in_=gm_ps)
            nvar = small.tile([GROUPS, 1], f32, tag="nvar")
            nc.vector.scalar_tensor_tensor(
                out=nvar, in0=gm_sb[:, 0:1], scalar=gm_sb[:, 0:1],
                in1=gm_sb[:, 1:2], op0=OP.mult, op1=OP.subtract)
            sq = small.tile([GROUPS, 1], f32, tag="sq")
            nc.scalar.activation(out=sq, in_=nvar, func=AF.Sqrt,
                                 bias=cst[0:GROUPS, _EPSC:_EPSC + 1],
                                 scale=-1.0)
            rsq = small.tile([GROUPS, 1], f32, tag="rsq")
            nc.vector.reciprocal(out=rsq, in_=sq)

            rm = small.tile([GROUPS, 2], f32, tag="rm")
            nc.vector.tensor_copy(out=rm[:, 0:1], in_=rsq)
            nc.vector.tensor_copy(out=rm[:, 1:2], in_=gm_sb[:, 0:1])
            bc_ps = spsum.tile([128, 2], f32, tag="s")
            nc.tensor.matmul(bc_ps, lhsT=gmask_t, rhs=rm, start=True, stop=True)
            bc_sb = small.tile([128, 2], f32, tag="bc")
            nc.vector.tensor_copy(out=bc_sb, in_=bc_ps)

            # alpha = rsq_o * gn_w ; beta = gn_b - alpha*(mu_o - out_b)
            alpha = small.tile([128, 1], f32, tag="alpha")
            nc.vector.tensor_scalar_mul(out=alpha, in0=bc_sb[:, 0:1],
                                        scalar1=gn_w)
            t3 = small.tile([128, 1], f32, tag="t3")
            nc.vector.scalar_tensor_tensor(out=t3, in0=bc_sb[:, 1:2],
                                           scalar=out_b, in1=alpha,
                                           op0=OP.subtract, op1=OP.mult)
            beta = small.tile([128, 1], f32, tag="beta")
            nc.vector.tensor_scalar(out=beta, in0=t3, scalar1=-1.0,
                                    scalar2=gn_b, op0=OP.mult, op1=OP.add)

            # ---- final affine, chunked so output DMA overlaps ScalarE
            for jc in range(4):
                sl = slice(jc * 512, (jc + 1) * 512)
                nc.scalar.activation(out=yn_sb[:, sl], in_=y_ps[:, sl],
                                     func=AF.Identity, bias=beta, scale=alpha)
                nc.sync.dma_start(out=out_d[:, sl], in_=yn_sb[:, sl])

    nc.compile()
    return nc


def _get_nc():
    global _NC
    if _NC is None:
        _NC = _build()
    return _NC


def _host_inputs(x, w_qkv, se_w1, se_b1, se_w2, se_b2, out_w, out_b,
                 gn_w, gn_b):
    import ml_dtypes
    bf = ml_dtypes.bfloat16
    f32 = np.float32

    def c(a, dt=f32):
        return np.ascontiguousarray(np.asarray(a), dtype=dt)

    x2 = np.asarray(x, dtype=f32).reshape(INNER, HW)
    w_qkv = np.asarray(w_qkv, dtype=f32)
    out_b = np.asarray(out_b, dtype=f32)

    wqkv = c(np.concatenate(
        [w_qkv[0:128].T, w_qkv[128:256].T, w_qkv[256:384].T], axis=1), bf)

    gs = 128 // GROUPS
    cst = np.zeros((128, NCONST), f32)
    cst[:, _IDENT[0]:_IDENT[1]] = np.eye(128, dtype=f32)
    cst[:, _OUTWT[0]:_OUTWT[1]] = np.asarray(out_w, dtype=f32).T
    cst[:, _SEW1T[0]:_SEW1T[1]] = np.asarray(se_w1, dtype=f32).T
    cst[0:32, _SEW2T[0]:_SEW2T[1]] = np.asarray(se_w2, dtype=f32).T
    cst[np.arange(128), _GMASK[0] + np.arange(128) // gs] = 1.0 / gs
    cst[np.arange(128) // gs, _GMASKT[0] + np.arange(128)] = 1.0
    cst[0:32, _SEB1] = np.asarray(se_b1, dtype=f32)
    cst[:, _SEB2] = np.asarray(se_b2, dtype=f32)
    cst[:, _OUTB] = out_b
    cst[:, _OUTB2X] = 2.0 * out_b
    cst[:, _OUTBSQ] = out_b * out_b
    cst[:, _GNW] = np.asarray(gn_w, dtype=f32)
    cst[:, _GNB] = np.asarray(gn_b, dtype=f32)
    cst[:, _ONESHW] = 1.0 / HW
    cst[:, _EPSC] = EPS

    in_maps = []
    for i in range(NCORES):
        in_maps.append({
            "x": c(x2[:, i * CHUNK:(i + 1) * CHUNK], bf),
            "wqkv": wqkv,
            "consts": cst,
        })
    return in_maps


def kernel(x, w_qkv, se_w1, se_b1, se_w2, se_b2, out_w, out_b, gn_w, gn_b):
    from concourse.bass_utils import run_bass_kernel_spmd

    in_maps = _host_inputs(x, w_qkv, se_w1, se_b1, se_w2, se_b2, out_w,
                           out_b, gn_w, gn_b)
    res = run_bass_kernel_spmd(_get_nc(), in_maps, core_ids=list(range(NCORES)))
    y = np.concatenate([np.asarray(res.results[i]["out"], dtype=np.float32)
                        for i in range(NCORES)], axis=1)
    B, C, H, W = 1, 128, 128, 128
    return y.reshape(B, C, H, W)
